# revision 2
# baseline (speedup 1.0000x reference)
"""Trainium2 Bass kernel for LoRACrossAttnProcessor (v2, bf16).

Strategy:
- Host: fold LoRA (W_eff = W + up @ down), fold attn scale into Wq, permute
  q/k/v channels into a head-packed layout (tiles 0-7 = head h channels
  0..127; tiles 8-9 = the 32-channel remainders of heads 0-3 / 4-7), and
  permute Wo columns to match.  Convert everything to bf16 (rel tol 2e-2).
- Shard: data-parallel over batch, 2 batch items per core, 8 cores.
- Device (per core, bf16 matmuls, fp32 PSUM):
    K.T = Wk_p @ E.T    [1280pack, 154]  (both batches packed in free dim)
    V   = E @ Wv_p.T    [77, 1280pack]   (natural, per batch)
    Q.T = Wq_p @ X.T    [1280pack, 1024] (per batch)
    per (batch, head, 512-seq-chunk):
      scores.T [77,512] = full-tile mm + zero-padded rem-tile mm (accum)
      exps = Exp(scores.T)                       (ACT, bf16 out)
      bc   [128,512] = ones77.T @ exps           (PE: every row = sumexp)
      at_full [128,512] = V_h_full.T @ exps      (PE)
      A.T tile h = at_full / bc                  (DVE STT divide, PSUM/PSUM)
    per (batch, rem-group, chunk): 4 zero-padded V-rem mms + 4 block-ones
      mms accumulate -> A.T tiles 8/9 = psar / psbr.
    O = A @ Wo_p.T  [1024, 1280]  natural layout (lhsT = A.T col-slices),
      streamed to DRAM as contiguous rows, bf16.
- Host: gather, upcast, add bo.
"""

import numpy as np
from contextlib import ExitStack

import ml_dtypes

import concourse.bass as bass
import concourse.mybir as mybir
import concourse.tile as tile
from concourse import bacc
from concourse.bass_utils import run_bass_kernel_spmd

F32 = mybir.dt.float32
BF16 = mybir.dt.bfloat16
AF = mybir.ActivationFunctionType
MULT = mybir.AluOpType.mult
DIV = mybir.AluOpType.divide

H = 8
B, S, C = 16, 1024, 1280
SENC, CENC = 77, 1024
D = C // H  # 160
NCORES = 8
BPC = B // NCORES  # 2 batches per core
P = 128
NCI_X = C // P      # 10 contraction tiles for Q/O proj
NCI_E = CENC // P   # 8 contraction tiles for K/V proj
NT = C // P         # 10 packed channel tiles
NST = S // 512      # 2 seq chunks of 512
E2 = BPC * SENC     # 154 packed encoder columns
ATTN_SCALE = 1.0 / float(np.sqrt(D))
NP_BF16 = ml_dtypes.bfloat16

# O-proj free-dim chunks (<=512 for one PSUM bank)
OCH = [(0, 512), (512, 512), (1024, 256)]


def build():
    nc = bacc.Bacc("TRN2", target_bir_lowering=False, debug=False)
    xt_d = nc.dram_tensor("xt", [BPC, C, S], BF16, kind="ExternalInput")
    et_d = nc.dram_tensor("et", [CENC, E2], BF16, kind="ExternalInput")
    wqt_d = nc.dram_tensor("wqt", [C, C], BF16, kind="ExternalInput")
    wkt_d = nc.dram_tensor("wkt", [CENC, C], BF16, kind="ExternalInput")
    wvt_d = nc.dram_tensor("wvt", [CENC, C], BF16, kind="ExternalInput")
    wot_d = nc.dram_tensor("wot", [C, C], BF16, kind="ExternalInput")
    out_d = nc.dram_tensor("out", [BPC, S, C], BF16, kind="ExternalOutput")

    with tile.TileContext(nc) as tc, ExitStack() as ctx:
        persist = ctx.enter_context(tc.tile_pool(name="persist", bufs=1))
        big = ctx.enter_context(tc.tile_pool(name="big", bufs=2))
        wqp = ctx.enter_context(tc.tile_pool(name="wqp", bufs=3))
        expp = ctx.enter_context(tc.tile_pool(name="expp", bufs=6))
        stag = ctx.enter_context(tc.tile_pool(name="stag", bufs=3))
        psum = ctx.enter_context(tc.tile_pool(name="psum", bufs=2, space="PSUM"))

        # ---- constants ----
        ones77 = persist.tile([SENC, P], BF16, tag="ones77")
        nc.vector.memset(ones77, 1.0)
        onesblk = []
        for j in range(4):
            t = persist.tile([SENC, P], BF16, tag=f"ob{j}", name=f"ob{j}")
            nc.vector.memset(t, 0.0)
            nc.vector.memset(t[:, 32 * j : 32 * j + 32], 1.0)
            onesblk.append(t)

        # ---- load E.T [1024, 154] -> [128, 8, 154] ----
        et_sb = persist.tile([P, NCI_E, E2], BF16, tag="et")
        nc.sync.dma_start(
            out=et_sb, in_=et_d.ap().rearrange("(ci p) k -> p ci k", p=P)
        )

        # ---- K.T proj: kt[t] [128, 154] bf16 ----
        wk_sb = big.tile([P, NCI_E, C], BF16, tag="big", name="wk")
        nc.sync.dma_start(
            out=wk_sb, in_=wkt_d.ap().rearrange("(ci p) c -> p ci c", p=P)
        )
        kt = []
        for t in range(NT):
            ps = psum.tile([P, 512], F32, tag="p")
            for ci in range(NCI_E):
                nc.tensor.matmul(
                    ps[:, :E2],
                    wk_sb[:, ci, t * P : (t + 1) * P],
                    et_sb[:, ci, :],
                    start=(ci == 0),
                    stop=(ci == NCI_E - 1),
                )
            ktt = persist.tile([P, E2], BF16, tag=f"kt{t}", name=f"kt{t}")
            nc.vector.tensor_copy(out=ktt, in_=ps[:, :E2])
            kt.append(ktt)
        # zero-padded rem K tiles: ktrem[h] rows 32j..+32 = kt[8+g] same rows
        ktrem = []
        for h in range(H):
            g, j = divmod(h, 4)
            t = persist.tile([P, E2], BF16, tag=f"ktr{h}", name=f"ktr{h}")
            nc.vector.memset(t, 0.0)
            nc.vector.tensor_copy(
                out=t[32 * j : 32 * j + 32, :],
                in_=kt[8 + g][32 * j : 32 * j + 32, :],
            )
            ktrem.append(t)

        # ---- V proj (natural, packed channels): v_sb[b] [77, 1280] ----
        wv_sb = big.tile([P, NCI_E, C], BF16, tag="big", name="wv")
        nc.sync.dma_start(
            out=wv_sb, in_=wvt_d.ap().rearrange("(ci p) c -> p ci c", p=P)
        )
        v_sb = []
        for b in range(BPC):
            v_sb.append(persist.tile([SENC, C], BF16, tag=f"v{b}", name=f"v{b}"))
        for b in range(BPC):
            for c0, cw in OCH:
                ps = psum.tile([P, 512], F32, tag="p")
                for ci in range(NCI_E):
                    nc.tensor.matmul(
                        ps[:SENC, :cw],
                        et_sb[:, ci, b * SENC : (b + 1) * SENC],
                        wv_sb[:, ci, c0 : c0 + cw],
                        start=(ci == 0),
                        stop=(ci == NCI_E - 1),
                    )
                nc.vector.tensor_copy(
                    out=v_sb[b][:, c0 : c0 + cw], in_=ps[:SENC, :cw]
                )
        # zero-padded rem V tiles: vrem[b][h] [77, 128], cols 32j..+32 live
        vrem = [[None] * H for _ in range(BPC)]
        for b in range(BPC):
            for h in range(H):
                g, j = divmod(h, 4)
                t = persist.tile(
                    [SENC, P], BF16, tag=f"vr{b}_{h}", name=f"vr{b}_{h}"
                )
                nc.vector.memset(t, 0.0)
                nc.vector.tensor_copy(
                    out=t[:, 32 * j : 32 * j + 32],
                    in_=v_sb[b][:, 1024 + 128 * g + 32 * j : 1024 + 128 * g + 32 * j + 32],
                )
                vrem[b][h] = t

        # ---- load X.T per batch [128, 10, 1024] ----
        xt_sb = []
        for b in range(BPC):
            t = persist.tile([P, NCI_X, S], BF16, tag=f"xt{b}", name=f"xt{b}")
            nc.sync.dma_start(
                out=t, in_=xt_d.ap()[b].rearrange("(ci p) s -> p ci s", p=P)
            )
            xt_sb.append(t)

        # ---- Q.T proj: qt[b] [128, 10, 1024] bf16 (wq streamed per tile) ----
        qt_sb = []
        for b in range(BPC):
            qt_sb.append(
                persist.tile([P, NT, S], BF16, tag=f"qt{b}", name=f"qt{b}")
            )
        for t in range(NT):
            wqb = wqp.tile([P, NCI_X, P], BF16, tag="wqb")
            nc.sync.dma_start(
                out=wqb,
                in_=wqt_d.ap()[:, t * P : (t + 1) * P].rearrange(
                    "(ci p) c -> p ci c", p=P
                ),
            )
            for b in range(BPC):
                for st in range(NST):
                    ps = psum.tile([P, 512], F32, tag="p")
                    for ci in range(NCI_X):
                        nc.tensor.matmul(
                            ps,
                            wqb[:, ci, :],
                            xt_sb[b][:, ci, st * 512 : st * 512 + 512],
                            start=(ci == 0),
                            stop=(ci == NCI_X - 1),
                        )
                    nc.vector.tensor_copy(
                        out=qt_sb[b][:, t, st * 512 : st * 512 + 512], in_=ps
                    )

        # ---- load Wo (full resident) [128, 10, 1280] ----
        wo_sb = persist.tile([P, NCI_X, C], BF16, tag="wo")
        nc.sync.dma_start(
            out=wo_sb, in_=wot_d.ap().rearrange("(ci p) c -> p ci c", p=P)
        )

        # ---- attention + O proj, per (b, st) ----
        at_sb = []
        for b in range(BPC):
            at_sb.append(big.tile([P, NT, S], BF16, tag="big", name=f"at{b}"))

        for b in range(BPC):
            for st in range(NST):
                sl = slice(st * 512, st * 512 + 512)
                exps_all = {}
                for h in range(H):
                    g = h // 4
                    # scores.T [77, 512]
                    ps_s = psum.tile([SENC, 512], F32, tag="sc")
                    nc.tensor.matmul(
                        ps_s,
                        kt[h][:, b * SENC : (b + 1) * SENC],
                        qt_sb[b][:, h, sl],
                        start=True,
                        stop=False,
                    )
                    nc.tensor.matmul(
                        ps_s,
                        ktrem[h][:, b * SENC : (b + 1) * SENC],
                        qt_sb[b][:, 8 + g, sl],
                        start=False,
                        stop=True,
                    )
                    exps = expp.tile([SENC, 512], BF16, tag="exps")
                    nc.scalar.activation(out=exps, in_=ps_s, func=AF.Exp)
                    exps_all[h] = exps
                    # bc [128, 512]: every row = sumexp_h
                    ps_b = psum.tile([P, 512], F32, tag="b")
                    nc.tensor.matmul(ps_b, ones77, exps, start=True, stop=True)
                    # at_full [128, 512]
                    ps_a = psum.tile([P, 512], F32, tag="a")
                    nc.tensor.matmul(
                        ps_a,
                        v_sb[b][:, P * h : P * h + P],
                        exps,
                        start=True,
                        stop=True,
                    )
                    nc.vector.scalar_tensor_tensor(
                        out=at_sb[b][:, h, sl],
                        in0=ps_a,
                        scalar=1.0,
                        in1=ps_b,
                        op0=MULT,
                        op1=DIV,
                    )
                    # rem group done once its 4 heads' exps exist
                    if h % 4 == 3:
                        ps_ar = psum.tile([P, 512], F32, tag="a")
                        ps_br = psum.tile([P, 512], F32, tag="b")
                        for j in range(4):
                            hh = 4 * g + j
                            nc.tensor.matmul(
                                ps_ar,
                                vrem[b][hh],
                                exps_all[hh],
                                start=(j == 0),
                                stop=(j == 3),
                            )
                            nc.tensor.matmul(
                                ps_br,
                                onesblk[j],
                                exps_all[hh],
                                start=(j == 0),
                                stop=(j == 3),
                            )
                        nc.vector.scalar_tensor_tensor(
                            out=at_sb[b][:, 8 + g, sl],
                            in0=ps_ar,
                            scalar=1.0,
                            in1=ps_br,
                            op0=MULT,
                            op1=DIV,
                        )
                # O proj for the 4 s-blocks of this (b, st)
                for sb in range(4):
                    sblk = st * 4 + sb
                    ot = stag.tile([P, C], BF16, tag="ot")
                    for c0, cw in OCH:
                        ps_o = psum.tile([P, 512], F32, tag="p")
                        for ci in range(NCI_X):
                            nc.tensor.matmul(
                                ps_o[:, :cw],
                                at_sb[b][:, ci, sblk * P : (sblk + 1) * P],
                                wo_sb[:, ci, c0 : c0 + cw],
                                start=(ci == 0),
                                stop=(ci == NCI_X - 1),
                            )
                        nc.scalar.copy(out=ot[:, c0 : c0 + cw], in_=ps_o[:, :cw])
                    nc.sync.dma_start(
                        out=out_d.ap()[b, sblk * P : (sblk + 1) * P, :], in_=ot
                    )

    nc.compile()
    return nc


_NC_CACHE = []


def _get_nc():
    if not _NC_CACHE:
        _NC_CACHE.append(build())
    return _NC_CACHE[0]


def _packed_perm():
    perm = np.zeros(C, np.int64)
    for h in range(H):
        perm[128 * h : 128 * h + 128] = 160 * h + np.arange(128)
    for g in range(2):
        for j in range(4):
            h = 4 * g + j
            p0 = 1024 + 128 * g + 32 * j
            perm[p0 : p0 + 32] = 160 * h + 128 + np.arange(32)
    return perm


def make_in_maps(hidden_states, encoder_hidden_states, Wq, Wk, Wv, Wo,
                 q_down, q_up, k_down, k_up, v_down, v_up, o_down, o_up):
    f8 = np.float64
    wq = Wq.astype(f8) + q_up.astype(f8) @ q_down.astype(f8)
    wk = Wk.astype(f8) + k_up.astype(f8) @ k_down.astype(f8)
    wv = Wv.astype(f8) + v_up.astype(f8) @ v_down.astype(f8)
    wo = Wo.astype(f8) + o_up.astype(f8) @ o_down.astype(f8)

    perm = _packed_perm()
    wqt = np.ascontiguousarray((wq[perm, :] * ATTN_SCALE).T).astype(NP_BF16)
    wkt = np.ascontiguousarray(wk[perm, :].T).astype(NP_BF16)
    wvt = np.ascontiguousarray(wv[perm, :].T).astype(NP_BF16)
    wot = np.ascontiguousarray(wo[:, perm].T).astype(NP_BF16)

    in_maps = []
    for c in range(NCORES):
        hs = hidden_states[c * BPC : (c + 1) * BPC]  # [2, S, C]
        xt = np.ascontiguousarray(hs.transpose(0, 2, 1)).astype(NP_BF16)
        enc = encoder_hidden_states[c * BPC : (c + 1) * BPC]  # [2, 77, 1024]
        et = np.concatenate([enc[b].T for b in range(BPC)], axis=1)
        et = np.ascontiguousarray(et).astype(NP_BF16)
        in_maps.append(
            {"xt": xt, "et": et, "wqt": wqt, "wkt": wkt, "wvt": wvt, "wot": wot}
        )
    return in_maps


def kernel(hidden_states, encoder_hidden_states, Wq, Wk, Wv, Wo, bo,
           q_down, q_up, k_down, k_up, v_down, v_up, o_down, o_up):
    nc = _get_nc()
    in_maps = make_in_maps(
        hidden_states, encoder_hidden_states, Wq, Wk, Wv, Wo,
        q_down, q_up, k_down, k_up, v_down, v_up, o_down, o_up,
    )
    res = run_bass_kernel_spmd(nc, in_maps, list(range(NCORES)))
    out = np.concatenate(
        [res.results[c]["out"].astype(np.float32) for c in range(NCORES)], axis=0
    )
    out = out + bo.astype(np.float32)[None, None, :]
    return out.astype(np.float32)


# revision 6
# speedup vs baseline: 26.8352x; 26.8352x over previous
"""Trainium2 Bass kernel for LoRACrossAttnProcessor (v2, bf16).

Strategy:
- Host: fold LoRA (W_eff = W + up @ down), fold attn scale into Wq, permute
  q/k/v channels into a head-packed layout (tiles 0-7 = head h channels
  0..127; tiles 8-9 = the 32-channel remainders of heads 0-3 / 4-7), and
  permute Wo columns to match.  Convert everything to bf16 (rel tol 2e-2).
- Shard: data-parallel over batch, 2 batch items per core, 8 cores.
- Device (per core, bf16 matmuls, fp32 PSUM):
    K.T = Wk_p @ E.T    [1280pack, 154]  (both batches packed in free dim)
    V   = E @ Wv_p.T    [77, 1280pack]   (natural, per batch)
    Q.T = Wq_p @ X.T    [1280pack, 1024] (per batch)
    per (batch, head, 512-seq-chunk):
      scores.T [77,512] = full-tile mm + zero-padded rem-tile mm (accum)
      exps = Exp(scores.T)                       (ACT, bf16 out)
      bc   [128,512] = ones77.T @ exps           (PE: every row = sumexp)
      at_full [128,512] = V_h_full.T @ exps      (PE)
      A.T tile h = at_full / bc                  (DVE STT divide, PSUM/PSUM)
    per (batch, rem-group, chunk): 4 zero-padded V-rem mms + 4 block-ones
      mms accumulate -> A.T tiles 8/9 = psar / psbr.
    O = A @ Wo_p.T  [1024, 1280]  natural layout (lhsT = A.T col-slices),
      streamed to DRAM as contiguous rows, bf16.
- Host: gather, upcast, add bo.
"""

import numpy as np
from contextlib import ExitStack

import ml_dtypes

import concourse.bass as bass
import concourse.mybir as mybir
import concourse.tile as tile
from concourse import bacc
from concourse.bass_utils import run_bass_kernel_spmd

F32 = mybir.dt.float32
BF16 = mybir.dt.bfloat16
AF = mybir.ActivationFunctionType
MULT = mybir.AluOpType.mult
DIV = mybir.AluOpType.divide

H = 8
B, S, C = 16, 1024, 1280
SENC, CENC = 77, 1024
D = C // H  # 160
NCORES = 8
BPC = B // NCORES  # 2 batches per core
P = 128
NCI_X = C // P      # 10 contraction tiles for Q/O proj
NCI_E = CENC // P   # 8 contraction tiles for K/V proj
NT = C // P         # 10 packed channel tiles
NST = S // 512      # 2 seq chunks of 512
E2 = BPC * SENC     # 154 packed encoder columns
ATTN_SCALE = 1.0 / float(np.sqrt(D))
NP_BF16 = ml_dtypes.bfloat16

# O-proj free-dim chunks (<=512 for one PSUM bank)
OCH = [(0, 512), (512, 512), (1024, 256)]


def build():
    nc = bacc.Bacc("TRN2", target_bir_lowering=False, debug=False)
    xt_d = nc.dram_tensor("xt", [BPC, C, S], BF16, kind="ExternalInput")
    et_d = nc.dram_tensor("et", [CENC, E2], BF16, kind="ExternalInput")
    wqt_d = nc.dram_tensor("wqt", [C, C], BF16, kind="ExternalInput")
    wkt_d = nc.dram_tensor("wkt", [CENC, C], BF16, kind="ExternalInput")
    wvt_d = nc.dram_tensor("wvt", [CENC, C], BF16, kind="ExternalInput")
    wot_d = nc.dram_tensor("wot", [C, C], BF16, kind="ExternalInput")
    out_d = nc.dram_tensor("out", [BPC, S, C], BF16, kind="ExternalOutput")

    with tile.TileContext(nc) as tc, ExitStack() as ctx:
        persist = ctx.enter_context(tc.tile_pool(name="persist", bufs=1))
        big = ctx.enter_context(tc.tile_pool(name="big", bufs=2))
        wqp = ctx.enter_context(tc.tile_pool(name="wqp", bufs=3))
        expp = ctx.enter_context(tc.tile_pool(name="expp", bufs=6))
        stag = ctx.enter_context(tc.tile_pool(name="stag", bufs=3))
        psum = ctx.enter_context(tc.tile_pool(name="psum", bufs=2, space="PSUM"))

        # ---- constants ----
        ones77 = persist.tile([SENC, P], BF16, tag="ones77")
        nc.vector.memset(ones77, 1.0)

        # ---- load E.T [1024, 154] -> [128, 8, 154] ----
        et_sb = persist.tile([P, NCI_E, E2], BF16, tag="et")
        nc.sync.dma_start(
            out=et_sb, in_=et_d.ap().rearrange("(ci p) k -> p ci k", p=P)
        )

        # ---- K.T proj: kt[t] [128, 154] bf16 ----
        wk_sb = big.tile([P, NCI_E, C], BF16, tag="big", name="wk")
        nc.sync.dma_start(
            out=wk_sb, in_=wkt_d.ap().rearrange("(ci p) c -> p ci c", p=P)
        )
        kt = []
        for t in range(NT):
            ps = psum.tile([P, 512], F32, tag="p")
            for ci in range(NCI_E):
                nc.tensor.matmul(
                    ps[:, :E2],
                    wk_sb[:, ci, t * P : (t + 1) * P],
                    et_sb[:, ci, :],
                    start=(ci == 0),
                    stop=(ci == NCI_E - 1),
                )
            ktt = persist.tile([P, E2], BF16, tag=f"kt{t}", name=f"kt{t}")
            nc.vector.tensor_copy(out=ktt, in_=ps[:, :E2])
            kt.append(ktt)
        # zero-padded rem K tiles: ktrem[h] rows 32j..+32 = kt[8+g] same rows
        ktrem = []
        for h in range(H):
            g, j = divmod(h, 4)
            t = persist.tile([P, E2], BF16, tag=f"ktr{h}", name=f"ktr{h}")
            nc.vector.memset(t, 0.0)
            nc.vector.tensor_copy(
                out=t[32 * j : 32 * j + 32, :],
                in_=kt[8 + g][32 * j : 32 * j + 32, :],
            )
            ktrem.append(t)

        # ---- V proj (natural, packed channels): v_sb[b] [77, 1280] ----
        wv_sb = big.tile([P, NCI_E, C], BF16, tag="big", name="wv")
        nc.sync.dma_start(
            out=wv_sb, in_=wvt_d.ap().rearrange("(ci p) c -> p ci c", p=P)
        )
        v_sb = []
        for b in range(BPC):
            v_sb.append(persist.tile([SENC, C], BF16, tag=f"v{b}", name=f"v{b}"))
        for b in range(BPC):
            for c0, cw in OCH:
                ps = psum.tile([P, 512], F32, tag="p")
                for ci in range(NCI_E):
                    nc.tensor.matmul(
                        ps[:SENC, :cw],
                        et_sb[:, ci, b * SENC : (b + 1) * SENC],
                        wv_sb[:, ci, c0 : c0 + cw],
                        start=(ci == 0),
                        stop=(ci == NCI_E - 1),
                    )
                nc.vector.tensor_copy(
                    out=v_sb[b][:, c0 : c0 + cw], in_=ps[:SENC, :cw]
                )
        # zero-padded rem V tiles: vrem[b][h] [77, 128], cols 32j..+32 live
        vrem = [[None] * H for _ in range(BPC)]
        for b in range(BPC):
            for h in range(H):
                g, j = divmod(h, 4)
                t = persist.tile(
                    [SENC, P], BF16, tag=f"vr{b}_{h}", name=f"vr{b}_{h}"
                )
                nc.vector.memset(t, 0.0)
                nc.vector.tensor_copy(
                    out=t[:, 32 * j : 32 * j + 32],
                    in_=v_sb[b][:, 1024 + 128 * g + 32 * j : 1024 + 128 * g + 32 * j + 32],
                )
                vrem[b][h] = t

        # ---- load X.T per batch [128, 10, 1024] ----
        xt_sb = []
        for b in range(BPC):
            t = persist.tile([P, NCI_X, S], BF16, tag=f"xt{b}", name=f"xt{b}")
            nc.sync.dma_start(
                out=t, in_=xt_d.ap()[b].rearrange("(ci p) s -> p ci s", p=P)
            )
            xt_sb.append(t)

        # ---- Q.T proj: qt[b] [128, 10, 1024] bf16 (wq streamed per tile) ----
        qt_sb = []
        for b in range(BPC):
            qt_sb.append(
                persist.tile([P, NT, S], BF16, tag=f"qt{b}", name=f"qt{b}")
            )
        for t in range(NT):
            wqb = wqp.tile([P, NCI_X, P], BF16, tag="wqb")
            nc.sync.dma_start(
                out=wqb,
                in_=wqt_d.ap()[:, t * P : (t + 1) * P].rearrange(
                    "(ci p) c -> p ci c", p=P
                ),
            )
            for b in range(BPC):
                for st in range(NST):
                    ps = psum.tile([P, 512], F32, tag="p")
                    for ci in range(NCI_X):
                        nc.tensor.matmul(
                            ps,
                            wqb[:, ci, :],
                            xt_sb[b][:, ci, st * 512 : st * 512 + 512],
                            start=(ci == 0),
                            stop=(ci == NCI_X - 1),
                        )
                    nc.vector.tensor_copy(
                        out=qt_sb[b][:, t, st * 512 : st * 512 + 512], in_=ps
                    )

        # ---- load Wo (full resident) [128, 10, 1280] ----
        wo_sb = persist.tile([P, NCI_X, C], BF16, tag="wo")
        nc.sync.dma_start(
            out=wo_sb, in_=wot_d.ap().rearrange("(ci p) c -> p ci c", p=P)
        )

        # ---- attention + O proj, per (b, st) ----
        at_sb = []
        for b in range(BPC):
            at_sb.append(big.tile([P, NT, S], BF16, tag="big", name=f"at{b}"))

        for b in range(BPC):
            for st in range(NST):
                sl = slice(st * 512, st * 512 + 512)
                probs_all = {}
                for h in range(H):
                    g = h // 4
                    # scores.T [77, 512]
                    ps_s = psum.tile([SENC, 512], F32, tag="sc")
                    nc.tensor.matmul(
                        ps_s,
                        kt[h][:, b * SENC : (b + 1) * SENC],
                        qt_sb[b][:, h, sl],
                        start=True,
                        stop=False,
                    )
                    nc.tensor.matmul(
                        ps_s,
                        ktrem[h][:, b * SENC : (b + 1) * SENC],
                        qt_sb[b][:, 8 + g, sl],
                        start=False,
                        stop=True,
                    )
                    exps = expp.tile([SENC, 512], BF16, tag="exps")
                    nc.scalar.activation(out=exps, in_=ps_s, func=AF.Exp)
                    # sumexp broadcast over 77 partitions [77, 512]
                    ps_b = psum.tile([SENC, 512], F32, tag="b")
                    nc.tensor.matmul(
                        ps_b, ones77[:, :SENC], exps, start=True, stop=True
                    )
                    # reciprocal of the broadcast sum, then normalize
                    rec = expp.tile([SENC, 512], F32, tag="rec")
                    nc.vector.reciprocal(out=rec, in_=ps_b)
                    probs = expp.tile([SENC, 512], BF16, tag="probs")
                    nc.vector.scalar_tensor_tensor(
                        out=probs,
                        in0=exps,
                        scalar=1.0,
                        in1=rec,
                        op0=MULT,
                        op1=MULT,
                    )
                    probs_all[h] = probs
                    # at_full [128, 512]
                    ps_a = psum.tile([P, 512], F32, tag="a")
                    nc.tensor.matmul(
                        ps_a,
                        v_sb[b][:, P * h : P * h + P],
                        probs,
                        start=True,
                        stop=True,
                    )
                    if h % 2 == 0:
                        nc.vector.tensor_copy(out=at_sb[b][:, h, sl], in_=ps_a)
                    else:
                        nc.scalar.copy(out=at_sb[b][:, h, sl], in_=ps_a)
                    # rem group done once its 4 heads' probs exist
                    if h % 4 == 3:
                        ps_ar = psum.tile([P, 512], F32, tag="a")
                        for j in range(4):
                            hh = 4 * g + j
                            nc.tensor.matmul(
                                ps_ar,
                                vrem[b][hh],
                                probs_all[hh],
                                start=(j == 0),
                                stop=(j == 3),
                            )
                        nc.vector.tensor_copy(
                            out=at_sb[b][:, 8 + g, sl], in_=ps_ar
                        )
                # O proj for the 4 s-blocks of this (b, st)
                for sb in range(4):
                    sblk = st * 4 + sb
                    ot = stag.tile([P, C], BF16, tag="ot")
                    for c0, cw in OCH:
                        ps_o = psum.tile([P, 512], F32, tag="p")
                        for ci in range(NCI_X):
                            nc.tensor.matmul(
                                ps_o[:, :cw],
                                at_sb[b][:, ci, sblk * P : (sblk + 1) * P],
                                wo_sb[:, ci, c0 : c0 + cw],
                                start=(ci == 0),
                                stop=(ci == NCI_X - 1),
                            )
                        nc.scalar.copy(out=ot[:, c0 : c0 + cw], in_=ps_o[:, :cw])
                    nc.sync.dma_start(
                        out=out_d.ap()[b, sblk * P : (sblk + 1) * P, :], in_=ot
                    )

    nc.compile()
    return nc


_NC_CACHE = []


def _get_nc():
    if not _NC_CACHE:
        _NC_CACHE.append(build())
    return _NC_CACHE[0]


def _packed_perm():
    perm = np.zeros(C, np.int64)
    for h in range(H):
        perm[128 * h : 128 * h + 128] = 160 * h + np.arange(128)
    for g in range(2):
        for j in range(4):
            h = 4 * g + j
            p0 = 1024 + 128 * g + 32 * j
            perm[p0 : p0 + 32] = 160 * h + 128 + np.arange(32)
    return perm


def make_in_maps(hidden_states, encoder_hidden_states, Wq, Wk, Wv, Wo,
                 q_down, q_up, k_down, k_up, v_down, v_up, o_down, o_up):
    f8 = np.float64
    wq = Wq.astype(f8) + q_up.astype(f8) @ q_down.astype(f8)
    wk = Wk.astype(f8) + k_up.astype(f8) @ k_down.astype(f8)
    wv = Wv.astype(f8) + v_up.astype(f8) @ v_down.astype(f8)
    wo = Wo.astype(f8) + o_up.astype(f8) @ o_down.astype(f8)

    perm = _packed_perm()
    wqt = np.ascontiguousarray((wq[perm, :] * ATTN_SCALE).T).astype(NP_BF16)
    wkt = np.ascontiguousarray(wk[perm, :].T).astype(NP_BF16)
    wvt = np.ascontiguousarray(wv[perm, :].T).astype(NP_BF16)
    wot = np.ascontiguousarray(wo[:, perm].T).astype(NP_BF16)

    in_maps = []
    for c in range(NCORES):
        hs = hidden_states[c * BPC : (c + 1) * BPC]  # [2, S, C]
        xt = np.ascontiguousarray(hs.transpose(0, 2, 1)).astype(NP_BF16)
        enc = encoder_hidden_states[c * BPC : (c + 1) * BPC]  # [2, 77, 1024]
        et = np.concatenate([enc[b].T for b in range(BPC)], axis=1)
        et = np.ascontiguousarray(et).astype(NP_BF16)
        in_maps.append(
            {"xt": xt, "et": et, "wqt": wqt, "wkt": wkt, "wvt": wvt, "wot": wot}
        )
    return in_maps


def kernel(hidden_states, encoder_hidden_states, Wq, Wk, Wv, Wo, bo,
           q_down, q_up, k_down, k_up, v_down, v_up, o_down, o_up):
    nc = _get_nc()
    in_maps = make_in_maps(
        hidden_states, encoder_hidden_states, Wq, Wk, Wv, Wo,
        q_down, q_up, k_down, k_up, v_down, v_up, o_down, o_up,
    )
    res = run_bass_kernel_spmd(nc, in_maps, list(range(NCORES)))
    out = np.concatenate(
        [res.results[c]["out"].astype(np.float32) for c in range(NCORES)], axis=0
    )
    out = out + bo.astype(np.float32)[None, None, :]
    return out.astype(np.float32)


# revision 8
# speedup vs baseline: 32.0505x; 1.1943x over previous
"""Trainium2 Bass kernel for LoRACrossAttnProcessor (v3, bf16).

Strategy:
- Host: fold LoRA (W_eff = W + up @ down), fold attn scale into Wq, permute
  q/k/v channels into a head-packed layout (tiles 0-7 = head h channels
  0..127; tiles 8-9 = the 32-channel remainders of heads 0-3 / 4-7), and
  permute Wo columns to match.  Convert everything to bf16 (rel tol 2e-2).
- Shard: data-parallel over batch, 2 batch items per core, 8 cores.
- Device (per core, bf16 matmuls, fp32 PSUM):
    K.T = Wk_p @ E.T    [1280pack, 154]  (both batches packed in free dim)
    V   = E @ Wv_p.T    [77, 1280pack]   (natural, per batch)
    Q.T = Wq_p @ X.T    [1280pack, 1024] (per batch; tiles 8,9 first so
                                          attention can start early)
    per (batch, head, 512-seq-chunk):
      scores.T [77,512] = full-tile mm + zero-padded rem-tile mm (accum)
      exps  = Exp(scores.T)                     (ACT, bf16 out)
      sum   [77,512] = ones.T @ exps            (PE broadcast-sum)
      rec   = reciprocal_approx_fast(sum)       (DVE custom op)
      probs = exps * rec                        (GpSimd STT, SBUF only)
      A.T tile h = V_h_full.T @ probs           (PE) -> copy (DVE/ACT)
    rem groups: 4 zero-padded V-rem mms accumulate -> A.T tiles 8/9.
    O = A @ Wo_p.T natural layout, staged 2 s-blocks per DMA, second HWDGE
      queue (scalar) for wo/xt1 loads + output stores.
- Host: gather, upcast, add bo.
"""

import numpy as np
from contextlib import ExitStack

import ml_dtypes

import concourse.bass as bass
import concourse.mybir as mybir
import concourse.tile as tile
from concourse import bacc
from concourse.bass_utils import run_bass_kernel_spmd

F32 = mybir.dt.float32
BF16 = mybir.dt.bfloat16
AF = mybir.ActivationFunctionType
MULT = mybir.AluOpType.mult

H = 8
B, S, C = 16, 1024, 1280
SENC, CENC = 77, 1024
D = C // H  # 160
NCORES = 8
BPC = B // NCORES  # 2 batches per core
P = 128
NCI_X = C // P      # 10 contraction tiles for Q/O proj
NCI_E = CENC // P   # 8 contraction tiles for K/V proj
NT = C // P         # 10 packed channel tiles
NST = S // 512      # 2 seq chunks of 512
E2 = BPC * SENC     # 154 packed encoder columns
ATTN_SCALE = 1.0 / float(np.sqrt(D))
NP_BF16 = ml_dtypes.bfloat16

T_ORDER = [8, 9] + list(range(8))  # q tiles 8,9 first: attention needs them
OCH = [(0, 512), (512, 512), (1024, 256)]


def build():
    nc = bacc.Bacc("TRN2", target_bir_lowering=False, debug=False)
    xt_d = nc.dram_tensor("xt", [BPC, C, S], BF16, kind="ExternalInput")
    et_d = nc.dram_tensor("et", [CENC, E2], BF16, kind="ExternalInput")
    wqt_d = nc.dram_tensor("wqt", [C, C], BF16, kind="ExternalInput")
    wkt_d = nc.dram_tensor("wkt", [CENC, C], BF16, kind="ExternalInput")
    wvt_d = nc.dram_tensor("wvt", [CENC, C], BF16, kind="ExternalInput")
    wot_d = nc.dram_tensor("wot", [C, C], BF16, kind="ExternalInput")
    out_d = nc.dram_tensor("out", [BPC, S, C], BF16, kind="ExternalOutput")

    with tile.TileContext(nc) as tc, ExitStack() as ctx:
        persist = ctx.enter_context(tc.tile_pool(name="persist", bufs=1))
        big = ctx.enter_context(tc.tile_pool(name="big", bufs=2))
        wqp = ctx.enter_context(tc.tile_pool(name="wqp", bufs=3))
        expp = ctx.enter_context(tc.tile_pool(name="expp", bufs=3))
        probp = ctx.enter_context(tc.tile_pool(name="probp", bufs=6))
        recp = ctx.enter_context(tc.tile_pool(name="recp", bufs=2))
        stag = ctx.enter_context(tc.tile_pool(name="stag", bufs=2))
        psum = ctx.enter_context(tc.tile_pool(name="psum", bufs=2, space="PSUM"))

        # ---- constants ----
        ones77 = persist.tile([SENC, P], BF16, tag="ones77")
        nc.vector.memset(ones77, 1.0)

        # ---- input DMAs, hand-ordered across the two HWDGE queues ----
        # sync queue: et, wk, xt0_a, wq8, wq9, xt0_b, wv, wq blocks (in loop)
        # scalar queue: xt1, wo, (outputs later)
        et_sb = persist.tile([P, NCI_E, E2], BF16, tag="et")
        nc.sync.dma_start(
            out=et_sb, in_=et_d.ap().rearrange("(ci p) k -> p ci k", p=P)
        )
        wk_sb = []
        for c in range(2):
            t = big.tile([P, 4, C], BF16, tag="big", name=f"wk{c}")
            nc.sync.dma_start(
                out=t,
                in_=wkt_d.ap()[512 * c : 512 * (c + 1), :].rearrange(
                    "(ci p) c -> p ci c", p=P
                ),
            )
            wk_sb.append(t)
        xt_sb = [[None, None], [None, None]]
        for c in range(2):
            t = persist.tile([P, 5, S], BF16, tag=f"xt0_{c}", name=f"xt0_{c}")
            nc.sync.dma_start(
                out=t,
                in_=xt_d.ap()[0, 640 * c : 640 * (c + 1), :].rearrange(
                    "(ci p) s -> p ci s", p=P
                ),
            )
            xt_sb[0][c] = t
        wq_pre = {}
        for t in T_ORDER[:2]:
            wqb = wqp.tile([P, NCI_X, P], BF16, tag="wqb")
            nc.sync.dma_start(
                out=wqb,
                in_=wqt_d.ap()[:, t * P : (t + 1) * P].rearrange(
                    "(ci p) c -> p ci c", p=P
                ),
            )
            wq_pre[t] = wqb
        wv_sb = []
        for c in range(2):
            t = big.tile([P, 4, C], BF16, tag="big", name=f"wv{c}")
            nc.sync.dma_start(
                out=t,
                in_=wvt_d.ap()[512 * c : 512 * (c + 1), :].rearrange(
                    "(ci p) c -> p ci c", p=P
                ),
            )
            wv_sb.append(t)
        # scalar queue
        for c in range(2):
            t = persist.tile([P, 5, S], BF16, tag=f"xt1_{c}", name=f"xt1_{c}")
            nc.scalar.dma_start(
                out=t,
                in_=xt_d.ap()[1, 640 * c : 640 * (c + 1), :].rearrange(
                    "(ci p) s -> p ci s", p=P
                ),
            )
            xt_sb[1][c] = t
        wo_sb = []
        for c in range(2):
            t = persist.tile([P, 5, C], BF16, tag=f"wo{c}", name=f"wo{c}")
            nc.scalar.dma_start(
                out=t,
                in_=wot_d.ap()[640 * c : 640 * (c + 1), :].rearrange(
                    "(ci p) c -> p ci c", p=P
                ),
            )
            wo_sb.append(t)

        def xt_ap(b, ci, sl):
            return xt_sb[b][ci // 5][:, ci % 5, sl]

        def wo_ap(ci, cs):
            return wo_sb[ci // 5][:, ci % 5, cs]

        # ---- K.T proj: kt[t] [128, 154] bf16 ----
        kt = []
        for t in range(NT):
            ps = psum.tile([P, 512], F32, tag="p")
            for ci in range(NCI_E):
                nc.tensor.matmul(
                    ps[:, :E2],
                    wk_sb[ci // 4][:, ci % 4, t * P : (t + 1) * P],
                    et_sb[:, ci, :],
                    start=(ci == 0),
                    stop=(ci == NCI_E - 1),
                )
            ktt = persist.tile([P, E2], BF16, tag=f"kt{t}", name=f"kt{t}")
            nc.vector.tensor_copy(out=ktt, in_=ps[:, :E2])
            kt.append(ktt)
        # zero-padded rem K tiles
        ktrem = []
        for h in range(H):
            g, j = divmod(h, 4)
            t = persist.tile([P, E2], BF16, tag=f"ktr{h}", name=f"ktr{h}")
            nc.vector.memset(t, 0.0)
            nc.vector.tensor_copy(
                out=t[32 * j : 32 * j + 32, :],
                in_=kt[8 + g][32 * j : 32 * j + 32, :],
            )
            ktrem.append(t)

        # ---- V proj (natural, packed channels): v_sb[b] [77, 1280] ----
        v_sb = []
        for b in range(BPC):
            v_sb.append(persist.tile([SENC, C], BF16, tag=f"v{b}", name=f"v{b}"))
        for b in range(BPC):
            for c0, cw in OCH:
                ps = psum.tile([P, 512], F32, tag="p")
                for ci in range(NCI_E):
                    nc.tensor.matmul(
                        ps[:SENC, :cw],
                        et_sb[:, ci, b * SENC : (b + 1) * SENC],
                        wv_sb[ci // 4][:, ci % 4, c0 : c0 + cw],
                        start=(ci == 0),
                        stop=(ci == NCI_E - 1),
                    )
                nc.vector.tensor_copy(
                    out=v_sb[b][:, c0 : c0 + cw], in_=ps[:SENC, :cw]
                )
        # zero-padded rem V tiles
        vrem = [[None] * H for _ in range(BPC)]
        for b in range(BPC):
            for h in range(H):
                g, j = divmod(h, 4)
                t = persist.tile(
                    [SENC, P], BF16, tag=f"vr{b}_{h}", name=f"vr{b}_{h}"
                )
                nc.vector.memset(t, 0.0)
                nc.vector.tensor_copy(
                    out=t[:, 32 * j : 32 * j + 32],
                    in_=v_sb[b][:, 1024 + 128 * g + 32 * j : 1024 + 128 * g + 32 * j + 32],
                )
                vrem[b][h] = t

        # ---- Q.T proj: qt[b] [128, 10, 1024] bf16 (tiles 8,9 first) ----
        qt_sb = []
        for b in range(BPC):
            qt_sb.append(
                persist.tile([P, NT, S], BF16, tag=f"qt{b}", name=f"qt{b}")
            )
        for t in T_ORDER:
            if t in wq_pre:
                wqb = wq_pre[t]
            else:
                wqb = wqp.tile([P, NCI_X, P], BF16, tag="wqb")
                nc.sync.dma_start(
                    out=wqb,
                    in_=wqt_d.ap()[:, t * P : (t + 1) * P].rearrange(
                        "(ci p) c -> p ci c", p=P
                    ),
                )
            for b in range(BPC):
                for st in range(NST):
                    sl = slice(st * 512, st * 512 + 512)
                    ps = psum.tile([P, 512], F32, tag="p")
                    for ci in range(NCI_X):
                        nc.tensor.matmul(
                            ps,
                            wqb[:, ci, :],
                            xt_ap(b, ci, sl),
                            start=(ci == 0),
                            stop=(ci == NCI_X - 1),
                        )
                    nc.vector.tensor_copy(
                        out=qt_sb[b][:, t, sl], in_=ps
                    )

        # ---- attention + O proj, per (b, st) ----
        at_sb = []
        for b in range(BPC):
            at_sb.append(big.tile([P, NT, S], BF16, tag="big", name=f"at{b}"))

        for b in range(BPC):
            for st in range(NST):
                sl = slice(st * 512, st * 512 + 512)
                probs_all = {}
                for h in range(H):
                    g = h // 4
                    # scores.T [77, 512]
                    ps_s = psum.tile([SENC, 512], F32, tag="sc")
                    nc.tensor.matmul(
                        ps_s,
                        kt[h][:, b * SENC : (b + 1) * SENC],
                        qt_sb[b][:, h, sl],
                        start=True,
                        stop=False,
                    )
                    nc.tensor.matmul(
                        ps_s,
                        ktrem[h][:, b * SENC : (b + 1) * SENC],
                        qt_sb[b][:, 8 + g, sl],
                        start=False,
                        stop=True,
                    )
                    exps = expp.tile([SENC, 512], BF16, tag="exps")
                    nc.scalar.activation(out=exps, in_=ps_s, func=AF.Exp)
                    # sumexp broadcast over 77 partitions [77, 512]
                    ps_b = psum.tile([SENC, 512], F32, tag="b")
                    nc.tensor.matmul(
                        ps_b, ones77[:, :SENC], exps, start=True, stop=True
                    )
                    rec = recp.tile([SENC, 512], F32, tag="rec")
                    nc.vector.reciprocal_approx_fast(out=rec, in_=ps_b)
                    probs = probp.tile([SENC, 512], BF16, tag="probs")
                    nc.vector.scalar_tensor_tensor(
                        out=probs,
                        in0=exps,
                        scalar=1.0,
                        in1=rec,
                        op0=MULT,
                        op1=MULT,
                    )
                    probs_all[h] = probs
                    # at_full [128, 512]
                    ps_a = psum.tile([P, 512], F32, tag="a")
                    nc.tensor.matmul(
                        ps_a,
                        v_sb[b][:, P * h : P * h + P],
                        probs,
                        start=True,
                        stop=True,
                    )
                    if h % 2 == 0:
                        nc.vector.tensor_copy(out=at_sb[b][:, h, sl], in_=ps_a)
                    else:
                        nc.scalar.copy(out=at_sb[b][:, h, sl], in_=ps_a)
                    # rem group done once its 4 heads' probs exist
                    if h % 4 == 3:
                        ps_ar = psum.tile([P, 512], F32, tag="a")
                        for j in range(4):
                            hh = 4 * g + j
                            nc.tensor.matmul(
                                ps_ar,
                                vrem[b][hh],
                                probs_all[hh],
                                start=(j == 0),
                                stop=(j == 3),
                            )
                        nc.vector.tensor_copy(
                            out=at_sb[b][:, 8 + g, sl], in_=ps_ar
                        )
                # O proj for the 4 s-blocks of this (b, st), staged in pairs
                for half in range(2):
                    ot = stag.tile([P, 2, C], BF16, tag="ot")
                    for k in range(2):
                        sblk = st * 4 + half * 2 + k
                        for c0, cw in OCH:
                            ps_o = psum.tile([P, 512], F32, tag="p")
                            for ci in range(NCI_X):
                                nc.tensor.matmul(
                                    ps_o[:, :cw],
                                    at_sb[b][:, ci, sblk * P : (sblk + 1) * P],
                                    wo_ap(ci, slice(c0, c0 + cw)),
                                    start=(ci == 0),
                                    stop=(ci == NCI_X - 1),
                                )
                            nc.scalar.copy(
                                out=ot[:, k, c0 : c0 + cw], in_=ps_o[:, :cw]
                            )
                    r0 = (st * 4 + half * 2) * P
                    nc.scalar.dma_start(
                        out=out_d.ap()[b, r0 : r0 + 2 * P, :].rearrange(
                            "(a p) c -> p a c", p=P
                        ),
                        in_=ot,
                    )

    nc.compile()
    return nc


_NC_CACHE = []


def _get_nc():
    if not _NC_CACHE:
        _NC_CACHE.append(build())
    return _NC_CACHE[0]


def _packed_perm():
    perm = np.zeros(C, np.int64)
    for h in range(H):
        perm[128 * h : 128 * h + 128] = 160 * h + np.arange(128)
    for g in range(2):
        for j in range(4):
            h = 4 * g + j
            p0 = 1024 + 128 * g + 32 * j
            perm[p0 : p0 + 32] = 160 * h + 128 + np.arange(32)
    return perm


def make_in_maps(hidden_states, encoder_hidden_states, Wq, Wk, Wv, Wo,
                 q_down, q_up, k_down, k_up, v_down, v_up, o_down, o_up):
    f8 = np.float64
    wq = Wq.astype(f8) + q_up.astype(f8) @ q_down.astype(f8)
    wk = Wk.astype(f8) + k_up.astype(f8) @ k_down.astype(f8)
    wv = Wv.astype(f8) + v_up.astype(f8) @ v_down.astype(f8)
    wo = Wo.astype(f8) + o_up.astype(f8) @ o_down.astype(f8)

    perm = _packed_perm()
    wqt = np.ascontiguousarray((wq[perm, :] * ATTN_SCALE).T).astype(NP_BF16)
    wkt = np.ascontiguousarray(wk[perm, :].T).astype(NP_BF16)
    wvt = np.ascontiguousarray(wv[perm, :].T).astype(NP_BF16)
    wot = np.ascontiguousarray(wo[:, perm].T).astype(NP_BF16)

    in_maps = []
    for c in range(NCORES):
        hs = hidden_states[c * BPC : (c + 1) * BPC]  # [2, S, C]
        xt = np.ascontiguousarray(hs.transpose(0, 2, 1)).astype(NP_BF16)
        enc = encoder_hidden_states[c * BPC : (c + 1) * BPC]  # [2, 77, 1024]
        et = np.concatenate([enc[b].T for b in range(BPC)], axis=1)
        et = np.ascontiguousarray(et).astype(NP_BF16)
        in_maps.append(
            {"xt": xt, "et": et, "wqt": wqt, "wkt": wkt, "wvt": wvt, "wot": wot}
        )
    return in_maps


def kernel(hidden_states, encoder_hidden_states, Wq, Wk, Wv, Wo, bo,
           q_down, q_up, k_down, k_up, v_down, v_up, o_down, o_up):
    nc = _get_nc()
    in_maps = make_in_maps(
        hidden_states, encoder_hidden_states, Wq, Wk, Wv, Wo,
        q_down, q_up, k_down, k_up, v_down, v_up, o_down, o_up,
    )
    res = run_bass_kernel_spmd(nc, in_maps, list(range(NCORES)))
    out = np.concatenate(
        [res.results[c]["out"].astype(np.float32) for c in range(NCORES)], axis=0
    )
    out = out + bo.astype(np.float32)[None, None, :]
    return out.astype(np.float32)


# revision 15
# speedup vs baseline: 32.2289x; 1.0056x over previous
"""Trainium2 Bass kernel for LoRACrossAttnProcessor (v3, bf16).

Strategy:
- Host: fold LoRA (W_eff = W + up @ down), fold attn scale into Wq, permute
  q/k/v channels into a head-packed layout (tiles 0-7 = head h channels
  0..127; tiles 8-9 = the 32-channel remainders of heads 0-3 / 4-7), and
  permute Wo columns to match.  Convert everything to bf16 (rel tol 2e-2).
- Shard: data-parallel over batch, 2 batch items per core, 8 cores.
- Device (per core, bf16 matmuls, fp32 PSUM):
    K.T = Wk_p @ E.T    [1280pack, 154]  (both batches packed in free dim)
    V   = E @ Wv_p.T    [77, 1280pack]   (natural, per batch)
    Q.T = Wq_p @ X.T    [1280pack, 1024] (per batch; tiles 8,9 first so
                                          attention can start early)
    per (batch, head, 512-seq-chunk):
      scores.T [77,512] = full-tile mm + zero-padded rem-tile mm (accum)
      exps  = Exp(scores.T)                     (ACT, bf16 out)
      sum   [77,512] = ones.T @ exps            (PE broadcast-sum)
      rec   = reciprocal_approx_fast(sum)       (DVE custom op)
      probs = exps * rec                        (GpSimd STT, SBUF only)
      A.T tile h = V_h_full.T @ probs           (PE) -> copy (DVE/ACT)
    rem groups: 4 zero-padded V-rem mms accumulate -> A.T tiles 8/9.
    O = A @ Wo_p.T natural layout, staged 2 s-blocks per DMA, second HWDGE
      queue (scalar) for wo/xt1 loads + output stores.
- Host: gather, upcast, add bo.
"""

import numpy as np
from contextlib import ExitStack

import ml_dtypes

import concourse.bass as bass
import concourse.mybir as mybir
import concourse.tile as tile
from concourse import bacc
from concourse.bass_utils import run_bass_kernel_spmd

F32 = mybir.dt.float32
BF16 = mybir.dt.bfloat16
AF = mybir.ActivationFunctionType
MULT = mybir.AluOpType.mult

H = 8
B, S, C = 16, 1024, 1280
SENC, CENC = 77, 1024
D = C // H  # 160
NCORES = 8
BPC = B // NCORES  # 2 batches per core
P = 128
NCI_X = C // P      # 10 contraction tiles for Q/O proj
NCI_E = CENC // P   # 8 contraction tiles for K/V proj
NT = C // P         # 10 packed channel tiles
NST = S // 512      # 2 seq chunks of 512
E2 = BPC * SENC     # 154 packed encoder columns
EPAD = 256          # et padded to 256 cols (512B DMA segments)
ATTN_SCALE = 1.0 / float(np.sqrt(D))
NP_BF16 = ml_dtypes.bfloat16

T_ORDER = [8, 9] + list(range(8))  # q tiles 8,9 first: attention needs them
OCH = [(0, 512), (512, 512), (1024, 256)]


def build():
    nc = bacc.Bacc("TRN2", target_bir_lowering=False, debug=False)
    xt_d = nc.dram_tensor("xt", [BPC, C, S], BF16, kind="ExternalInput")
    et_d = nc.dram_tensor("et", [CENC, EPAD], BF16, kind="ExternalInput")
    wqt_d = nc.dram_tensor("wqt", [C, C], BF16, kind="ExternalInput")
    wkt_d = nc.dram_tensor("wkt", [CENC, C], BF16, kind="ExternalInput")
    wvt_d = nc.dram_tensor("wvt", [CENC, C], BF16, kind="ExternalInput")
    wot_d = nc.dram_tensor("wot", [C, C], BF16, kind="ExternalInput")
    out_d = nc.dram_tensor("out", [BPC, S, C], BF16, kind="ExternalOutput")

    with tile.TileContext(nc) as tc, ExitStack() as ctx:
        persist = ctx.enter_context(tc.tile_pool(name="persist", bufs=1))
        big = ctx.enter_context(tc.tile_pool(name="big", bufs=2))
        wqp = ctx.enter_context(tc.tile_pool(name="wqp", bufs=3))
        expp = ctx.enter_context(tc.tile_pool(name="expp", bufs=3))
        probp = ctx.enter_context(tc.tile_pool(name="probp", bufs=6))
        recp = ctx.enter_context(tc.tile_pool(name="recp", bufs=2))
        stag = ctx.enter_context(tc.tile_pool(name="stag", bufs=2))
        psum = ctx.enter_context(tc.tile_pool(name="psum", bufs=2, space="PSUM"))

        # ---- constants ----
        ones77 = persist.tile([SENC, P], BF16, tag="ones77")
        nc.vector.memset(ones77, 1.0)

        # ---- input DMAs, hand-ordered across the two HWDGE queues ----
        # sync queue:   xt0_a, wq8, xt0_b, wq9, et, wk, wq blocks (in loop)
        # scalar queue: xt1, wv, wo, (outputs later)
        xt_sb = [[None, None], [None, None]]
        wq_pre = {}

        t = persist.tile([P, 5, S], BF16, tag="xt0_0", name="xt0_0")
        nc.sync.dma_start(
            out=t,
            in_=xt_d.ap()[0, 0:640, :].rearrange("(ci p) s -> p ci s", p=P),
        )
        xt_sb[0][0] = t
        wqb = wqp.tile([P, NCI_X, P], BF16, tag="wqb")
        nc.sync.dma_start(
            out=wqb,
            in_=wqt_d.ap()[:, 8 * P : 9 * P].rearrange("(ci p) c -> p ci c", p=P),
        )
        wq_pre[8] = wqb
        t = persist.tile([P, 5, S], BF16, tag="xt0_1", name="xt0_1")
        nc.sync.dma_start(
            out=t,
            in_=xt_d.ap()[0, 640:1280, :].rearrange("(ci p) s -> p ci s", p=P),
        )
        xt_sb[0][1] = t
        wqb = wqp.tile([P, NCI_X, P], BF16, tag="wqb")
        nc.sync.dma_start(
            out=wqb,
            in_=wqt_d.ap()[:, 9 * P : 10 * P].rearrange("(ci p) c -> p ci c", p=P),
        )
        wq_pre[9] = wqb
        et_sb = persist.tile([P, NCI_E, EPAD], BF16, tag="et")
        nc.sync.dma_start(
            out=et_sb, in_=et_d.ap().rearrange("(ci p) k -> p ci k", p=P)
        )
        wk_sb = []
        for c in range(2):
            t = big.tile([P, 4, C], BF16, tag="big", name=f"wk{c}")
            nc.sync.dma_start(
                out=t,
                in_=wkt_d.ap()[512 * c : 512 * (c + 1), :].rearrange(
                    "(ci p) c -> p ci c", p=P
                ),
            )
            wk_sb.append(t)
        # scalar queue
        for c in range(2):
            t = persist.tile([P, 5, S], BF16, tag=f"xt1_{c}", name=f"xt1_{c}")
            nc.scalar.dma_start(
                out=t,
                in_=xt_d.ap()[1, 640 * c : 640 * (c + 1), :].rearrange(
                    "(ci p) s -> p ci s", p=P
                ),
            )
            xt_sb[1][c] = t
        wv_sb = []
        for c in range(2):
            t = big.tile([P, 4, C], BF16, tag="big", name=f"wv{c}")
            nc.scalar.dma_start(
                out=t,
                in_=wvt_d.ap()[512 * c : 512 * (c + 1), :].rearrange(
                    "(ci p) c -> p ci c", p=P
                ),
            )
            wv_sb.append(t)
        wo_sb = []
        for c in range(2):
            t = persist.tile([P, 5, C], BF16, tag=f"wo{c}", name=f"wo{c}")
            nc.scalar.dma_start(
                out=t,
                in_=wot_d.ap()[640 * c : 640 * (c + 1), :].rearrange(
                    "(ci p) c -> p ci c", p=P
                ),
            )
            wo_sb.append(t)

        def xt_ap(b, ci, sl):
            return xt_sb[b][ci // 5][:, ci % 5, sl]

        def wo_ap(ci, cs):
            return wo_sb[ci // 5][:, ci % 5, cs]

        # ---- Q.T proj helper (emitted per packed tile t) ----
        qt_sb = []
        for b in range(BPC):
            qt_sb.append(
                persist.tile([P, NT, S], BF16, tag=f"qt{b}", name=f"qt{b}")
            )

        def qproj_tile(t):
            if t in wq_pre:
                wqb = wq_pre[t]
            else:
                wqb = wqp.tile([P, NCI_X, P], BF16, tag="wqb")
                nc.sync.dma_start(
                    out=wqb,
                    in_=wqt_d.ap()[:, t * P : (t + 1) * P].rearrange(
                        "(ci p) c -> p ci c", p=P
                    ),
                )
            for b in range(BPC):
                for st in range(NST):
                    sl = slice(st * 512, st * 512 + 512)
                    ps = psum.tile([P, 512], F32, tag="p")
                    for ci in range(NCI_X):
                        nc.tensor.matmul(
                            ps,
                            wqb[:, ci, :],
                            xt_ap(b, ci, sl),
                            start=(ci == 0),
                            stop=(ci == NCI_X - 1),
                        )
                    nc.vector.tensor_copy(out=qt_sb[b][:, t, sl], in_=ps)

        # Q proj tiles 8, 9 first: every attention head needs them, and their
        # inputs are first in the DMA queues.
        qproj_tile(8)
        qproj_tile(9)

        # ---- K.T proj: kt[t] [128, 154] bf16 ----
        kt = []
        for t in range(NT):
            ps = psum.tile([P, 512], F32, tag="p")
            for ci in range(NCI_E):
                nc.tensor.matmul(
                    ps[:, :E2],
                    wk_sb[ci // 4][:, ci % 4, t * P : (t + 1) * P],
                    et_sb[:, ci, :E2],
                    start=(ci == 0),
                    stop=(ci == NCI_E - 1),
                )
            ktt = persist.tile([P, E2], BF16, tag=f"kt{t}", name=f"kt{t}")
            nc.vector.tensor_copy(out=ktt, in_=ps[:, :E2])
            kt.append(ktt)
        # zero-padded rem K tiles
        ktrem = []
        for h in range(H):
            g, j = divmod(h, 4)
            t = persist.tile([P, E2], BF16, tag=f"ktr{h}", name=f"ktr{h}")
            nc.vector.memset(t, 0.0)
            nc.vector.tensor_copy(
                out=t[32 * j : 32 * j + 32, :],
                in_=kt[8 + g][32 * j : 32 * j + 32, :],
            )
            ktrem.append(t)

        # ---- V proj (natural, packed channels): v_sb[b] [77, 1280] ----
        v_sb = []
        for b in range(BPC):
            v_sb.append(persist.tile([SENC, C], BF16, tag=f"v{b}", name=f"v{b}"))
        for b in range(BPC):
            for c0, cw in OCH:
                ps = psum.tile([P, 512], F32, tag="p")
                for ci in range(NCI_E):
                    nc.tensor.matmul(
                        ps[:SENC, :cw],
                        et_sb[:, ci, b * SENC : (b + 1) * SENC],
                        wv_sb[ci // 4][:, ci % 4, c0 : c0 + cw],
                        start=(ci == 0),
                        stop=(ci == NCI_E - 1),
                    )
                nc.vector.tensor_copy(
                    out=v_sb[b][:, c0 : c0 + cw], in_=ps[:SENC, :cw]
                )
        # zero-padded rem V tiles
        vrem = [[None] * H for _ in range(BPC)]
        for b in range(BPC):
            for h in range(H):
                g, j = divmod(h, 4)
                t = persist.tile(
                    [SENC, P], BF16, tag=f"vr{b}_{h}", name=f"vr{b}_{h}"
                )
                nc.vector.memset(t, 0.0)
                nc.vector.tensor_copy(
                    out=t[:, 32 * j : 32 * j + 32],
                    in_=v_sb[b][:, 1024 + 128 * g + 32 * j : 1024 + 128 * g + 32 * j + 32],
                )
                vrem[b][h] = t

        # ---- Q.T proj: remaining tiles 0..7 ----
        for t in range(8):
            qproj_tile(t)

        # ---- attention + O proj, per (b, st) ----
        at_sb = []
        for b in range(BPC):
            at_sb.append(big.tile([P, NT, S], BF16, tag="big", name=f"at{b}"))

        for b in range(BPC):
            for st in range(NST):
                sl = slice(st * 512, st * 512 + 512)
                probs_all = {}
                for h in range(H):
                    g = h // 4
                    # scores.T [77, 512]
                    ps_s = psum.tile([SENC, 512], F32, tag="sc")
                    nc.tensor.matmul(
                        ps_s,
                        kt[h][:, b * SENC : (b + 1) * SENC],
                        qt_sb[b][:, h, sl],
                        start=True,
                        stop=False,
                    )
                    nc.tensor.matmul(
                        ps_s,
                        ktrem[h][:, b * SENC : (b + 1) * SENC],
                        qt_sb[b][:, 8 + g, sl],
                        start=False,
                        stop=True,
                    )
                    exps = expp.tile([SENC, 512], BF16, tag="exps")
                    nc.scalar.activation(out=exps, in_=ps_s, func=AF.Exp)
                    # sumexp broadcast over 77 partitions [77, 512]
                    ps_b = psum.tile([SENC, 512], F32, tag="b")
                    nc.tensor.matmul(
                        ps_b, ones77[:, :SENC], exps, start=True, stop=True
                    )
                    rec = recp.tile([SENC, 512], F32, tag="rec")
                    nc.vector.reciprocal_approx_fast(out=rec, in_=ps_b)
                    probs = probp.tile([SENC, 512], BF16, tag="probs")
                    nc.vector.scalar_tensor_tensor(
                        out=probs,
                        in0=exps,
                        scalar=1.0,
                        in1=rec,
                        op0=MULT,
                        op1=MULT,
                    )
                    probs_all[h] = probs
                    # at_full [128, 512]
                    ps_a = psum.tile([P, 512], F32, tag="a")
                    nc.tensor.matmul(
                        ps_a,
                        v_sb[b][:, P * h : P * h + P],
                        probs,
                        start=True,
                        stop=True,
                    )
                    if h % 2 == 0:
                        nc.vector.tensor_copy(out=at_sb[b][:, h, sl], in_=ps_a)
                    else:
                        nc.scalar.copy(out=at_sb[b][:, h, sl], in_=ps_a)
                    # rem group done once its 4 heads' probs exist
                    if h % 4 == 3:
                        ps_ar = psum.tile([P, 512], F32, tag="a")
                        for j in range(4):
                            hh = 4 * g + j
                            nc.tensor.matmul(
                                ps_ar,
                                vrem[b][hh],
                                probs_all[hh],
                                start=(j == 0),
                                stop=(j == 3),
                            )
                        nc.vector.tensor_copy(
                            out=at_sb[b][:, 8 + g, sl], in_=ps_ar
                        )
                # O proj for the 4 s-blocks of this (b, st), staged in pairs
                for half in range(2):
                    ot = stag.tile([P, 2, C], BF16, tag="ot")
                    for k in range(2):
                        sblk = st * 4 + half * 2 + k
                        for c0, cw in OCH:
                            ps_o = psum.tile([P, 512], F32, tag="p")
                            for ci in range(NCI_X):
                                nc.tensor.matmul(
                                    ps_o[:, :cw],
                                    at_sb[b][:, ci, sblk * P : (sblk + 1) * P],
                                    wo_ap(ci, slice(c0, c0 + cw)),
                                    start=(ci == 0),
                                    stop=(ci == NCI_X - 1),
                                )
                            nc.scalar.copy(
                                out=ot[:, k, c0 : c0 + cw], in_=ps_o[:, :cw]
                            )
                    r0 = (st * 4 + half * 2) * P
                    nc.scalar.dma_start(
                        out=out_d.ap()[b, r0 : r0 + 2 * P, :].rearrange(
                            "(a p) c -> p a c", p=P
                        ),
                        in_=ot,
                    )

    nc.compile()
    return nc


_NC_CACHE = []


def _get_nc():
    if not _NC_CACHE:
        _NC_CACHE.append(build())
    return _NC_CACHE[0]


def _packed_perm():
    perm = np.zeros(C, np.int64)
    for h in range(H):
        perm[128 * h : 128 * h + 128] = 160 * h + np.arange(128)
    for g in range(2):
        for j in range(4):
            h = 4 * g + j
            p0 = 1024 + 128 * g + 32 * j
            perm[p0 : p0 + 32] = 160 * h + 128 + np.arange(32)
    return perm


def make_in_maps(hidden_states, encoder_hidden_states, Wq, Wk, Wv, Wo,
                 q_down, q_up, k_down, k_up, v_down, v_up, o_down, o_up):
    f8 = np.float64
    wq = Wq.astype(f8) + q_up.astype(f8) @ q_down.astype(f8)
    wk = Wk.astype(f8) + k_up.astype(f8) @ k_down.astype(f8)
    wv = Wv.astype(f8) + v_up.astype(f8) @ v_down.astype(f8)
    wo = Wo.astype(f8) + o_up.astype(f8) @ o_down.astype(f8)

    perm = _packed_perm()
    wqt = np.ascontiguousarray((wq[perm, :] * ATTN_SCALE).T).astype(NP_BF16)
    wkt = np.ascontiguousarray(wk[perm, :].T).astype(NP_BF16)
    wvt = np.ascontiguousarray(wv[perm, :].T).astype(NP_BF16)
    wot = np.ascontiguousarray(wo[:, perm].T).astype(NP_BF16)

    in_maps = []
    for c in range(NCORES):
        hs = hidden_states[c * BPC : (c + 1) * BPC]  # [2, S, C]
        xt = np.ascontiguousarray(hs.transpose(0, 2, 1)).astype(NP_BF16)
        enc = encoder_hidden_states[c * BPC : (c + 1) * BPC]  # [2, 77, 1024]
        et = np.zeros((CENC, EPAD), NP_BF16)
        et[:, :E2] = np.concatenate(
            [enc[b].T for b in range(BPC)], axis=1
        ).astype(NP_BF16)
        in_maps.append(
            {"xt": xt, "et": et, "wqt": wqt, "wkt": wkt, "wvt": wvt, "wot": wot}
        )
    return in_maps


def kernel(hidden_states, encoder_hidden_states, Wq, Wk, Wv, Wo, bo,
           q_down, q_up, k_down, k_up, v_down, v_up, o_down, o_up):
    nc = _get_nc()
    in_maps = make_in_maps(
        hidden_states, encoder_hidden_states, Wq, Wk, Wv, Wo,
        q_down, q_up, k_down, k_up, v_down, v_up, o_down, o_up,
    )
    res = run_bass_kernel_spmd(nc, in_maps, list(range(NCORES)))
    out = np.concatenate(
        [res.results[c]["out"].astype(np.float32) for c in range(NCORES)], axis=0
    )
    out = out + bo.astype(np.float32)[None, None, :]
    return out.astype(np.float32)


# revision 17
# speedup vs baseline: 33.3916x; 1.0361x over previous
"""Trainium2 Bass kernel for LoRACrossAttnProcessor (v3, bf16).

Strategy:
- Host: fold LoRA (W_eff = W + up @ down), fold attn scale into Wq, permute
  q/k/v channels into a head-packed layout (tiles 0-7 = head h channels
  0..127; tiles 8-9 = the 32-channel remainders of heads 0-3 / 4-7), and
  permute Wo columns to match.  Convert everything to bf16 (rel tol 2e-2).
- Shard: data-parallel over batch, 2 batch items per core, 8 cores.
- Device (per core, bf16 matmuls, fp32 PSUM):
    K.T = Wk_p @ E.T    [1280pack, 154]  (both batches packed in free dim)
    V   = E @ Wv_p.T    [77, 1280pack]   (natural, per batch)
    Q.T = Wq_p @ X.T    [1280pack, 1024] (per batch; tiles 8,9 first so
                                          attention can start early)
    per (batch, head, 512-seq-chunk):
      scores.T [77,512] = full-tile mm + zero-padded rem-tile mm (accum)
      exps  = Exp(scores.T)                     (ACT, bf16 out)
      sum   [77,512] = ones.T @ exps            (PE broadcast-sum)
      rec   = reciprocal_approx_fast(sum)       (DVE custom op)
      probs = exps * rec                        (GpSimd STT, SBUF only)
      A.T tile h = V_h_full.T @ probs           (PE) -> copy (DVE/ACT)
    rem groups: 4 zero-padded V-rem mms accumulate -> A.T tiles 8/9.
    O = A @ Wo_p.T natural layout, staged 2 s-blocks per DMA, second HWDGE
      queue (scalar) for wo/xt1 loads + output stores.
- Host: gather, upcast, add bo.
"""

import numpy as np
from contextlib import ExitStack

import ml_dtypes

import concourse.bass as bass
import concourse.mybir as mybir
import concourse.tile as tile
from concourse import bacc
from concourse.bass_utils import run_bass_kernel_spmd

F32 = mybir.dt.float32
BF16 = mybir.dt.bfloat16
AF = mybir.ActivationFunctionType
MULT = mybir.AluOpType.mult

H = 8
B, S, C = 16, 1024, 1280
SENC, CENC = 77, 1024
D = C // H  # 160
NCORES = 8
BPC = B // NCORES  # 2 batches per core
P = 128
NCI_X = C // P      # 10 contraction tiles for Q/O proj
NCI_E = CENC // P   # 8 contraction tiles for K/V proj
NT = C // P         # 10 packed channel tiles
NST = S // 512      # 2 seq chunks of 512
E2 = BPC * SENC     # 154 packed encoder columns
EPAD = 256          # et padded to 256 cols (512B DMA segments)
ATTN_SCALE = 1.0 / float(np.sqrt(D))
NP_BF16 = ml_dtypes.bfloat16

T_ORDER = [8, 9] + list(range(8))  # q tiles 8,9 first: attention needs them
OCH = [(0, 512), (512, 512), (1024, 256)]


def build():
    nc = bacc.Bacc("TRN2", target_bir_lowering=False, debug=False)
    xt_d = nc.dram_tensor("xt", [BPC, P, NCI_X, S], BF16, kind="ExternalInput")
    et_d = nc.dram_tensor("et", [P, NCI_E, EPAD], BF16, kind="ExternalInput")
    wqt_d = nc.dram_tensor("wqt", [NT, P, NCI_X, P], BF16, kind="ExternalInput")
    wkt_d = nc.dram_tensor("wkt", [P, NCI_E, C], BF16, kind="ExternalInput")
    wvt_d = nc.dram_tensor("wvt", [P, NCI_E, C], BF16, kind="ExternalInput")
    wot_d = nc.dram_tensor("wot", [P, NCI_X, C], BF16, kind="ExternalInput")
    out_d = nc.dram_tensor("out", [BPC, S, C], BF16, kind="ExternalOutput")

    with tile.TileContext(nc) as tc, ExitStack() as ctx:
        persist = ctx.enter_context(tc.tile_pool(name="persist", bufs=1))
        big = ctx.enter_context(tc.tile_pool(name="big", bufs=2))
        wqp = ctx.enter_context(tc.tile_pool(name="wqp", bufs=3))
        expp = ctx.enter_context(tc.tile_pool(name="expp", bufs=3))
        probp = ctx.enter_context(tc.tile_pool(name="probp", bufs=6))
        recp = ctx.enter_context(tc.tile_pool(name="recp", bufs=2))
        stag = ctx.enter_context(tc.tile_pool(name="stag", bufs=2))
        psum = ctx.enter_context(tc.tile_pool(name="psum", bufs=2, space="PSUM"))

        # ---- constants ----
        ones77 = persist.tile([SENC, P], BF16, tag="ones77")
        nc.vector.memset(ones77, 1.0)

        # ---- input DMAs, hand-ordered across the two HWDGE queues ----
        # sync queue:   xt0_a, wq8, xt0_b, wq9, et, wk, wq blocks (in loop)
        # scalar queue: xt1, wv, wo, (outputs later)
        xt_sb = [[None, None], [None, None]]
        wq_pre = {}

        t = persist.tile([P, 5, S], BF16, tag="xt0_0", name="xt0_0")
        nc.sync.dma_start(
            out=t,
            in_=xt_d.ap()[0, :, 0:5, :],
        )
        xt_sb[0][0] = t
        wqb = wqp.tile([P, NCI_X, P], BF16, tag="wqb")
        nc.sync.dma_start(
            out=wqb,
            in_=wqt_d.ap()[8],
        )
        wq_pre[8] = wqb
        t = persist.tile([P, 5, S], BF16, tag="xt0_1", name="xt0_1")
        nc.sync.dma_start(
            out=t,
            in_=xt_d.ap()[0, :, 5:10, :],
        )
        xt_sb[0][1] = t
        wqb = wqp.tile([P, NCI_X, P], BF16, tag="wqb")
        nc.sync.dma_start(
            out=wqb,
            in_=wqt_d.ap()[9],
        )
        wq_pre[9] = wqb
        et_sb = persist.tile([P, NCI_E, EPAD], BF16, tag="et")
        nc.sync.dma_start(
            out=et_sb, in_=et_d.ap()
        )
        wk_sb = []
        for c in range(2):
            t = big.tile([P, 4, C], BF16, tag="big", name=f"wk{c}")
            nc.sync.dma_start(
                out=t,
                in_=wkt_d.ap()[:, 4 * c : 4 * (c + 1), :],
            )
            wk_sb.append(t)
        # scalar queue
        for c in range(2):
            t = persist.tile([P, 5, S], BF16, tag=f"xt1_{c}", name=f"xt1_{c}")
            nc.scalar.dma_start(
                out=t,
                in_=xt_d.ap()[1, :, 5 * c : 5 * (c + 1), :],
            )
            xt_sb[1][c] = t
        wv_sb = []
        for c in range(2):
            t = big.tile([P, 4, C], BF16, tag="big", name=f"wv{c}")
            nc.scalar.dma_start(
                out=t,
                in_=wvt_d.ap()[:, 4 * c : 4 * (c + 1), :],
            )
            wv_sb.append(t)
        wo_sb = []
        for c in range(2):
            t = persist.tile([P, 5, C], BF16, tag=f"wo{c}", name=f"wo{c}")
            nc.scalar.dma_start(
                out=t,
                in_=wot_d.ap()[:, 5 * c : 5 * (c + 1), :],
            )
            wo_sb.append(t)

        def xt_ap(b, ci, sl):
            return xt_sb[b][ci // 5][:, ci % 5, sl]

        def wo_ap(ci, cs):
            return wo_sb[ci // 5][:, ci % 5, cs]

        # ---- Q.T proj helper (emitted per packed tile t) ----
        qt_sb = []
        for b in range(BPC):
            qt_sb.append(
                persist.tile([P, NT, S], BF16, tag=f"qt{b}", name=f"qt{b}")
            )

        def qproj_tile(t):
            if t in wq_pre:
                wqb = wq_pre[t]
            else:
                wqb = wqp.tile([P, NCI_X, P], BF16, tag="wqb")
                nc.sync.dma_start(
                    out=wqb,
                    in_=wqt_d.ap()[t],
                )
            for b in range(BPC):
                for st in range(NST):
                    sl = slice(st * 512, st * 512 + 512)
                    ps = psum.tile([P, 512], F32, tag="p")
                    for ci in range(NCI_X):
                        nc.tensor.matmul(
                            ps,
                            wqb[:, ci, :],
                            xt_ap(b, ci, sl),
                            start=(ci == 0),
                            stop=(ci == NCI_X - 1),
                        )
                    nc.vector.tensor_copy(out=qt_sb[b][:, t, sl], in_=ps)

        # Q proj tiles 8, 9 first: every attention head needs them, and their
        # inputs are first in the DMA queues.
        qproj_tile(8)
        qproj_tile(9)

        # ---- K.T proj: kt[t] [128, 154] bf16 ----
        kt = []
        for t in range(NT):
            ps = psum.tile([P, 512], F32, tag="p")
            for ci in range(NCI_E):
                nc.tensor.matmul(
                    ps[:, :E2],
                    wk_sb[ci // 4][:, ci % 4, t * P : (t + 1) * P],
                    et_sb[:, ci, :E2],
                    start=(ci == 0),
                    stop=(ci == NCI_E - 1),
                )
            ktt = persist.tile([P, E2], BF16, tag=f"kt{t}", name=f"kt{t}")
            nc.vector.tensor_copy(out=ktt, in_=ps[:, :E2])
            kt.append(ktt)
        # zero-padded rem K tiles
        ktrem = []
        for h in range(H):
            g, j = divmod(h, 4)
            t = persist.tile([P, E2], BF16, tag=f"ktr{h}", name=f"ktr{h}")
            nc.vector.memset(t, 0.0)
            nc.vector.tensor_copy(
                out=t[32 * j : 32 * j + 32, :],
                in_=kt[8 + g][32 * j : 32 * j + 32, :],
            )
            ktrem.append(t)

        # ---- V proj (natural, packed channels): v_sb[b] [77, 1280] ----
        v_sb = []
        for b in range(BPC):
            v_sb.append(persist.tile([SENC, C], BF16, tag=f"v{b}", name=f"v{b}"))
        for b in range(BPC):
            for c0, cw in OCH:
                ps = psum.tile([P, 512], F32, tag="p")
                for ci in range(NCI_E):
                    nc.tensor.matmul(
                        ps[:SENC, :cw],
                        et_sb[:, ci, b * SENC : (b + 1) * SENC],
                        wv_sb[ci // 4][:, ci % 4, c0 : c0 + cw],
                        start=(ci == 0),
                        stop=(ci == NCI_E - 1),
                    )
                nc.vector.tensor_copy(
                    out=v_sb[b][:, c0 : c0 + cw], in_=ps[:SENC, :cw]
                )
        # zero-padded rem V tiles
        vrem = [[None] * H for _ in range(BPC)]
        for b in range(BPC):
            for h in range(H):
                g, j = divmod(h, 4)
                t = persist.tile(
                    [SENC, P], BF16, tag=f"vr{b}_{h}", name=f"vr{b}_{h}"
                )
                nc.vector.memset(t, 0.0)
                nc.vector.tensor_copy(
                    out=t[:, 32 * j : 32 * j + 32],
                    in_=v_sb[b][:, 1024 + 128 * g + 32 * j : 1024 + 128 * g + 32 * j + 32],
                )
                vrem[b][h] = t

        # ---- Q.T proj: remaining tiles 0..7 ----
        for t in range(8):
            qproj_tile(t)

        # ---- attention + O proj, per (b, st) ----
        at_sb = []
        for b in range(BPC):
            at_sb.append(big.tile([P, NT, S], BF16, tag="big", name=f"at{b}"))

        for b in range(BPC):
            for st in range(NST):
                sl = slice(st * 512, st * 512 + 512)
                probs_all = {}
                for h in range(H):
                    g = h // 4
                    # scores.T [77, 512]
                    ps_s = psum.tile([SENC, 512], F32, tag="sc")
                    nc.tensor.matmul(
                        ps_s,
                        kt[h][:, b * SENC : (b + 1) * SENC],
                        qt_sb[b][:, h, sl],
                        start=True,
                        stop=False,
                    )
                    nc.tensor.matmul(
                        ps_s,
                        ktrem[h][:, b * SENC : (b + 1) * SENC],
                        qt_sb[b][:, 8 + g, sl],
                        start=False,
                        stop=True,
                    )
                    exps = expp.tile([SENC, 512], BF16, tag="exps")
                    nc.scalar.activation(out=exps, in_=ps_s, func=AF.Exp)
                    # sumexp broadcast over 77 partitions [77, 512]
                    ps_b = psum.tile([SENC, 512], F32, tag="b")
                    nc.tensor.matmul(
                        ps_b, ones77[:, :SENC], exps, start=True, stop=True
                    )
                    rec = recp.tile([SENC, 512], F32, tag="rec")
                    nc.vector.reciprocal_approx_fast(out=rec, in_=ps_b)
                    probs = probp.tile([SENC, 512], BF16, tag="probs")
                    nc.vector.scalar_tensor_tensor(
                        out=probs,
                        in0=exps,
                        scalar=1.0,
                        in1=rec,
                        op0=MULT,
                        op1=MULT,
                    )
                    probs_all[h] = probs
                    # at_full [128, 512]
                    ps_a = psum.tile([P, 512], F32, tag="a")
                    nc.tensor.matmul(
                        ps_a,
                        v_sb[b][:, P * h : P * h + P],
                        probs,
                        start=True,
                        stop=True,
                    )
                    if h % 2 == 0:
                        nc.vector.tensor_copy(out=at_sb[b][:, h, sl], in_=ps_a)
                    else:
                        nc.scalar.copy(out=at_sb[b][:, h, sl], in_=ps_a)
                    # rem group done once its 4 heads' probs exist
                    if h % 4 == 3:
                        ps_ar = psum.tile([P, 512], F32, tag="a")
                        for j in range(4):
                            hh = 4 * g + j
                            nc.tensor.matmul(
                                ps_ar,
                                vrem[b][hh],
                                probs_all[hh],
                                start=(j == 0),
                                stop=(j == 3),
                            )
                        nc.vector.tensor_copy(
                            out=at_sb[b][:, 8 + g, sl], in_=ps_ar
                        )
                # O proj for the 4 s-blocks of this (b, st), staged in pairs
                for half in range(2):
                    ot = stag.tile([P, 2, C], BF16, tag="ot")
                    for k in range(2):
                        sblk = st * 4 + half * 2 + k
                        for c0, cw in OCH:
                            ps_o = psum.tile([P, 512], F32, tag="p")
                            for ci in range(NCI_X):
                                nc.tensor.matmul(
                                    ps_o[:, :cw],
                                    at_sb[b][:, ci, sblk * P : (sblk + 1) * P],
                                    wo_ap(ci, slice(c0, c0 + cw)),
                                    start=(ci == 0),
                                    stop=(ci == NCI_X - 1),
                                )
                            nc.scalar.copy(
                                out=ot[:, k, c0 : c0 + cw], in_=ps_o[:, :cw]
                            )
                    r0 = (st * 4 + half * 2) * P
                    nc.scalar.dma_start(
                        out=out_d.ap()[b, r0 : r0 + 2 * P, :].rearrange(
                            "(a p) c -> p a c", p=P
                        ),
                        in_=ot,
                    )

    nc.compile()
    return nc


_NC_CACHE = []


def _get_nc():
    if not _NC_CACHE:
        _NC_CACHE.append(build())
    return _NC_CACHE[0]


def _packed_perm():
    perm = np.zeros(C, np.int64)
    for h in range(H):
        perm[128 * h : 128 * h + 128] = 160 * h + np.arange(128)
    for g in range(2):
        for j in range(4):
            h = 4 * g + j
            p0 = 1024 + 128 * g + 32 * j
            perm[p0 : p0 + 32] = 160 * h + 128 + np.arange(32)
    return perm


def make_in_maps(hidden_states, encoder_hidden_states, Wq, Wk, Wv, Wo,
                 q_down, q_up, k_down, k_up, v_down, v_up, o_down, o_up):
    f8 = np.float64
    wq = Wq.astype(f8) + q_up.astype(f8) @ q_down.astype(f8)
    wk = Wk.astype(f8) + k_up.astype(f8) @ k_down.astype(f8)
    wv = Wv.astype(f8) + v_up.astype(f8) @ v_down.astype(f8)
    wo = Wo.astype(f8) + o_up.astype(f8) @ o_down.astype(f8)

    perm = _packed_perm()
    # device-friendly layouts: partition dim first, contiguous per partition
    wq2 = (wq[perm, :] * ATTN_SCALE).T  # [x-ch, packed-q]
    wqt = np.ascontiguousarray(
        wq2.reshape(NCI_X, P, NT, P).transpose(2, 1, 0, 3)
    ).astype(NP_BF16)  # [t, p, ci, co]
    wkt = np.ascontiguousarray(
        wk[perm, :].T.reshape(NCI_E, P, C).transpose(1, 0, 2)
    ).astype(NP_BF16)  # [p, ci, c]
    wvt = np.ascontiguousarray(
        wv[perm, :].T.reshape(NCI_E, P, C).transpose(1, 0, 2)
    ).astype(NP_BF16)
    wot = np.ascontiguousarray(
        wo[:, perm].T.reshape(NCI_X, P, C).transpose(1, 0, 2)
    ).astype(NP_BF16)

    in_maps = []
    for c in range(NCORES):
        hs = hidden_states[c * BPC : (c + 1) * BPC]  # [2, S, C]
        xt = np.stack(
            [
                hs[b].T.reshape(NCI_X, P, S).transpose(1, 0, 2)
                for b in range(BPC)
            ]
        )  # [b, p, ci, s]
        xt = np.ascontiguousarray(xt).astype(NP_BF16)
        enc = encoder_hidden_states[c * BPC : (c + 1) * BPC]  # [2, 77, 1024]
        etp = np.zeros((CENC, EPAD), np.float32)
        etp[:, :E2] = np.concatenate([enc[b].T for b in range(BPC)], axis=1)
        et = np.ascontiguousarray(
            etp.reshape(NCI_E, P, EPAD).transpose(1, 0, 2)
        ).astype(NP_BF16)  # [p, ci, k]
        in_maps.append(
            {"xt": xt, "et": et, "wqt": wqt, "wkt": wkt, "wvt": wvt, "wot": wot}
        )
    return in_maps


def kernel(hidden_states, encoder_hidden_states, Wq, Wk, Wv, Wo, bo,
           q_down, q_up, k_down, k_up, v_down, v_up, o_down, o_up):
    nc = _get_nc()
    in_maps = make_in_maps(
        hidden_states, encoder_hidden_states, Wq, Wk, Wv, Wo,
        q_down, q_up, k_down, k_up, v_down, v_up, o_down, o_up,
    )
    res = run_bass_kernel_spmd(nc, in_maps, list(range(NCORES)))
    out = np.concatenate(
        [res.results[c]["out"].astype(np.float32) for c in range(NCORES)], axis=0
    )
    out = out + bo.astype(np.float32)[None, None, :]
    return out.astype(np.float32)
